# revision 1
# baseline (speedup 1.0000x reference)
"""Trainium2 Bass kernel for DigitCapsuleLayer (single routing iteration).

Math: with num_iterations == 1 the routing coefficients are uniform 1/R, so

    v[b,c,o] = squash( (1/R) * sum_{r,i} x[b,r,i] * W[0,r,c,o,i] )

i.e. one big [B=128, K=32768] x [K=32768, N=1024] fp32 matmul followed by a
tiny squash nonlinearity.  W is 128 MB and read exactly once -> the kernel is
HBM-bound at ~144 MB of total traffic.

Sharding (8 cores): split the contraction dim K = (routes x incap) so each
core reads a distinct 16 MB slice of W (and a 2 MB slice of x) and computes a
[128, 1024] partial product.  The cross-core sum is done with AllToAll
collectives (each core collects the 8 partials for its 16-row batch slice and
sums them locally on the vector engine) -- much cheaper than ReduceScatter on
this runtime.  The output N dim is processed in two halves so the first
AllToAll (and the collective entry/rank-skew cost) hides under the second
half's DMA + matmul stream.  Each core applies the squash on its batch slice
and the host concatenates the 8 slices (pure data movement).
"""

import numpy as np

import concourse.bacc as bacc
import concourse.bass as bass
import concourse.bass_utils as bass_utils
import concourse.mybir as mybir
import concourse.tile as tile

# Problem shape (hardcoded per the kernel contract).
B, R, C, I, O = 128, 2048, 32, 16, 32
NCORES = 8
RSH = R // NCORES          # 256 routes per core
KS = RSH * I               # 4096 contraction rows per core
KC = KS // 128             # 32 k-chunks of 128
N = C * O                  # 1024
NH = N // 2                # 512 columns per half
BS = B // NCORES           # 16 batch rows per core after the exchange

# PE fp32 runs at 4 cycles/row; float32r streams at 1 cycle/row for N>=256
# with ~1e-4-level relative error.  Accumulation stays in fp32 PSUM.
USE_F32R = True
# W k-chunk DMA group sizes per half (sums to KC); small first group so the
# PE starts as early as possible.
W_GROUPS = [2, 6, 8, 8, 4, 2, 1, 1]
# Exchange partials in fp16: halves the AllToAll payload; the partials are
# O(0.1)-magnitude sums so fp16 adds only ~2e-4 relative error.
EXCH_DT_NP = "float16"


def _build_program():
    nc = bacc.Bacc(
        "TRN2", target_bir_lowering=False, debug=False, num_devices=NCORES
    )
    f32 = mybir.dt.float32
    mm_dt = mybir.dt.float32r if USE_F32R else mybir.dt.float32
    ex_dt = getattr(mybir.dt, EXCH_DT_NP)

    xT = nc.dram_tensor("xT", [128, KC * B], mm_dt, kind="ExternalInput").ap()
    # Half-major W so each half's stream is fully contiguous per partition.
    Wt = nc.dram_tensor("Wt", [2, 128, KC, NH], mm_dt, kind="ExternalInput").ap()
    out = nc.dram_tensor("out", [BS, N], f32, kind="ExternalOutput").ap()

    with tile.TileContext(nc) as tc:
        with (
            tc.tile_pool(name="xpool", bufs=1) as xpool,
            tc.tile_pool(name="wpool", bufs=1) as wpool,
            tc.tile_pool(name="spool", bufs=1) as spool,
            tc.tile_pool(name="qpool", bufs=1) as qpool,
            tc.tile_pool(name="psum", bufs=1, space="PSUM") as psum_pool,
            tc.tile_pool(name="dram", bufs=1, space="DRAM") as dram_pool,
        ):
            # Warm the Sqrt ACT table off the critical path.
            warm = qpool.tile([1, 1], f32)
            nc.vector.memset(warm[:], 0.0)
            nc.scalar.sqrt(warm[:], warm[:])

            # x slice resident in SBUF: [p=k%128, (kc, b)] = 2 MB, loaded in
            # 4 chunks interleaved ahead of the first W groups on the sync
            # ring so matmul kc can start as soon as its chunks land.
            x_sb = xpool.tile([128, KC * B], mm_dt)

            for h in range(2):
                # This half's W columns, all 32 k-chunks: [128, KC, 512] 8 MB.
                w_sb = wpool.tile(
                    [128, KC, NH], mm_dt, name=f"w_sb{h}", tag=f"w{h}"
                )
                # The sync ring carries ONLY the W/x streams (HWDGE rings are
                # FIFO per engine -- any dependent DMA here would stall it).
                g0 = 0
                for gi, gsz in enumerate(W_GROUPS):
                    if h == 0 and gi < 4:
                        xpart = KC * B // 4
                        nc.sync.dma_start(
                            x_sb[:, gi * xpart : (gi + 1) * xpart],
                            xT[:, gi * xpart : (gi + 1) * xpart],
                        )
                    nc.sync.dma_start(
                        w_sb[:, g0 : g0 + gsz, :],
                        Wt[h, :, g0 : g0 + gsz, :],
                    )
                    g0 += gsz

                ps = psum_pool.tile([128, NH], f32, name=f"ps{h}", tag=f"ps{h}")
                for kc in range(KC):
                    nc.tensor.matmul(
                        ps,
                        x_sb[:, kc * B : (kc + 1) * B],
                        w_sb[:, kc, :],
                        start=(kc == 0),
                        stop=(kc == KC - 1),
                    )

                # Scale partial by 1/R while copying PSUM -> SBUF (DVE),
                # casting to the exchange dtype.  Both halves land in ONE
                # [128, N] tile: half-major col order happens to equal the
                # natural (c, o) order since c = 16h + c_local.
                if h == 0:
                    s_sb = spool.tile([128, N], ex_dt, name="s_sb")
                    cc_in = dram_pool.tile([B, N], ex_dt, name="cc_in")
                nc.vector.tensor_scalar_mul(
                    s_sb[:, h * NH : (h + 1) * NH], ps[:], 1.0 / R
                )
                # Bounce each half out as soon as its scale lands so the
                # collective doorbell fires right after the last one.
                nc.gpsimd.dma_start(
                    cc_in[:, h * NH : (h + 1) * NH],
                    s_sb[:, h * NH : (h + 1) * NH],
                )

            # Exchange partials with a SINGLE AllToAll (a second collective
            # costs ~11 us of ncfw setup each; the first one is gated by the
            # slowest rank regardless).  After it, partition rows
            # [16j, 16j+16) of cc_out hold core j's partial for THIS core's
            # batch slice.  Bounce DMA rides the gpsimd (SWDGE) path -- the
            # HWDGE rings are FIFO and busy with W / loads.
            cc_out = dram_pool.tile([B, N], ex_dt, name="cc_out")
            nc.gpsimd.collective_compute(
                "AllToAll",
                mybir.AluOpType.bypass,
                replica_groups=[list(range(NCORES))],
                ins=[cc_in.opt()],
                outs=[cc_out.opt()],
            )

            # Sum the 8 partials and apply the squash.  Partition layout:
            # p = (b_local, ch) with ch = 8 chunks of 128 columns; within a
            # chunk f = (cl, o) with c = ch*4 + cl.
            # SBUF [p=(b,ch), j, fl=128]: per-(p,j) 256 B contiguous.
            s8 = qpool.tile([128, NCORES, 128], ex_dt, name="s8")
            nc.scalar.dma_start(
                s8[:],
                cc_out.rearrange(
                    "(j b) (ch fl) -> (b ch) j fl", j=NCORES, ch=8, fl=128
                ),
            )
            # Sum over j (stride-permuted read, j innermost).
            sv = qpool.tile([128, 128], f32, name="sv")
            nc.vector.reduce_sum(
                sv[:],
                s8[:].rearrange("p j fl -> p fl j"),
                axis=mybir.AxisListType.X,
            )
            # Sum of squares over o within each cl group: [128, 4].
            s2 = qpool.tile([128, 4, 32], f32, name="s2")
            nc.vector.tensor_mul(
                out=s2[:],
                in0=sv[:].rearrange("p (cl o) -> p cl o", o=32),
                in1=sv[:].rearrange("p (cl o) -> p cl o", o=32),
            )
            sq = qpool.tile([128, 4], f32, name="sq")
            nc.vector.reduce_sum(sq[:], s2[:], axis=mybir.AxisListType.X)
            rt = qpool.tile([128, 4], f32, name="rt")
            nc.scalar.sqrt(rt[:], sq[:])
            den = qpool.tile([128, 4], f32, name="den")
            nc.vector.tensor_scalar_add(den[:], sq[:], 1.0)
            rec = qpool.tile([128, 4], f32, name="rec")
            nc.vector.reciprocal(rec[:], den[:])
            fac = qpool.tile([128, 4], f32, name="fac")
            nc.vector.tensor_mul(out=fac[:], in0=rt[:], in1=rec[:])
            v = qpool.tile([128, 4, 32], f32, name="v")
            nc.vector.tensor_tensor(
                v[:],
                sv[:].rearrange("p (cl o) -> p cl o", o=32),
                fac[:, :, None].to_broadcast((128, 4, 32)),
                mybir.AluOpType.mult,
            )
            nc.scalar.dma_start(
                out.rearrange("b (ch fl) -> (b ch) fl", ch=8),
                v[:].rearrange("p cl o -> p (cl o)"),
            )

    nc.compile()
    return nc


def _shard_inputs(x: np.ndarray, W: np.ndarray):
    """Per-core input layouts (pure data movement on host).

    Contraction index within core m: k = kc*128 + p with p = (rp, i),
    rp in [0,8); global route r = m*256 + kc*8 + rp.
    """
    in_maps = []
    for m in range(NCORES):
        xm = x[:, m * RSH : (m + 1) * RSH, :]          # (b, rr, i)
        xm = xm.reshape(B, KC, 8, I)                   # (b, kc, rp, i)
        x_prep = np.ascontiguousarray(
            xm.transpose(2, 3, 1, 0)                   # (rp, i, kc, b)
        ).reshape(128, KC * B)

        Wm = W[0, m * RSH : (m + 1) * RSH]             # (rr, c, o, i)
        Wm = Wm.reshape(KC, 8, 2, C // 2, O, I)        # (kc, rp, h, cl16, o, i)
        w_prep = np.ascontiguousarray(
            Wm.transpose(2, 1, 5, 0, 3, 4)             # (h, rp, i, kc, cl16, o)
        ).reshape(2, 128, KC, NH)

        in_maps.append({"xT": x_prep, "Wt": w_prep})
    return in_maps


_CACHED_NC = None


def _get_nc():
    global _CACHED_NC
    if _CACHED_NC is None:
        _CACHED_NC = _build_program()
    return _CACHED_NC


def kernel(x: np.ndarray, W: np.ndarray, _trace: bool = False):
    x = np.ascontiguousarray(np.asarray(x, dtype=np.float32))
    W = np.ascontiguousarray(np.asarray(W, dtype=np.float32))
    nc = _get_nc()
    in_maps = _shard_inputs(x, W)
    res = bass_utils.run_bass_kernel_spmd(
        nc, in_maps, core_ids=list(range(NCORES)), trace=_trace
    )
    out = np.concatenate(
        [res.results[m]["out"] for m in range(NCORES)], axis=0
    ).reshape(B, C, O, 1)
    if _trace:
        return out, res
    return out



# revision 2
# speedup vs baseline: 1.8216x; 1.8216x over previous
"""Trainium2 Bass kernel for DigitCapsuleLayer (single routing iteration).

Math: with num_iterations == 1 the routing coefficients are uniform 1/R, so

    v[b,c,o] = squash( (1/R) * sum_{r,i} x[b,r,i] * W[0,r,c,o,i] )

i.e. one big [B=128, K=32768] x [K=32768, N=1024] matmul followed by a tiny
squash nonlinearity.  W is read exactly once -> the kernel is HBM-bound.

Sharding (8 cores): split the OUTPUT capsule dim C=32 so each core owns 4
capsules (128 output columns) and computes them completely locally:
it reads its 8 MB W slice plus the full x (8 MB, bf16) and accumulates all
256 k-chunks into one PSUM tile.  No cross-core collective at all -- the
profile of the previous (K-sharded + AllToAll) version showed the ncfw
collective machinery (entry-barrier rank skew + ~11 us setup + a 30 us
AllToAll for 256 KB + tail) burning ~73 us after a 63 us near-roofline
stream.  Trading 7 MB of extra per-core DMA (x replication) for zero
collective wins by a wide margin, and makes each core's span independent
of rank start-skew.

Inputs are cast to bf16 on the host: the harness tolerance is 2e-2 and the
bf16 rounding contributes only ~3e-4 relative error through the 32768-term
contraction (measured; fp8 would land at ~4e-2 -> not safe).  bf16 also
halves HBM traffic and streams through the PE at 1 cycle/row with FWL
weight loads (~81 ns per 128x128x128 LDW+MM pair, well under the ~185 ns/
chunk DMA arrival rate at the measured 346 GB/s).
"""

import numpy as np
import ml_dtypes

import concourse.bacc as bacc
import concourse.bass as bass
import concourse.bass_utils as bass_utils
import concourse.mybir as mybir
import concourse.tile as tile

# Problem shape (hardcoded per the kernel contract).
B, R, C, I, O = 128, 2048, 32, 16, 32
NCORES = 8
CSH = C // NCORES            # 4 capsules per core
NCOL = CSH * O               # 128 output columns per core
KC = (R * I) // 128          # 256 contraction chunks of 128
# Per-group kc counts for the interleaved x/W DMA stream (2 DMAs per group).
# Small first groups so the PE starts early; ~1.5-3 MB steady-state groups
# keep descriptor overhead amortized (>=1 MB -> ~78% of peak per transfer).
GROUPS = [4, 8, 16, 32, 48, 48, 48, 52]
assert sum(GROUPS) == KC

BF16 = ml_dtypes.bfloat16


def _build_program():
    nc = bacc.Bacc(
        "TRN2", target_bir_lowering=False, debug=False, num_devices=NCORES
    )
    f32 = mybir.dt.float32
    bf16 = mybir.dt.bfloat16

    xt = nc.dram_tensor("xt", [128, KC, B], bf16, kind="ExternalInput").ap()
    wt = nc.dram_tensor("wt", [128, KC, NCOL], bf16, kind="ExternalInput").ap()
    out = nc.dram_tensor("out", [B, NCOL], f32, kind="ExternalOutput").ap()

    with tile.TileContext(nc) as tc:
        with (
            tc.tile_pool(name="xpool", bufs=1) as xpool,
            tc.tile_pool(name="wpool", bufs=1) as wpool,
            tc.tile_pool(name="qpool", bufs=1) as qpool,
            tc.tile_pool(name="psum", bufs=1, space="PSUM") as psum_pool,
        ):
            # Warm the Sqrt ACT table off the critical path.
            warm = qpool.tile([1, 1], f32)
            nc.vector.memset(warm[:], 0.0)
            nc.scalar.sqrt(warm[:], warm[:])

            # Both streams fully SBUF-resident: 64 KB/partition each.
            x_sb = xpool.tile([128, KC, B], bf16)
            w_sb = wpool.tile([128, KC, NCOL], bf16)
            g0 = 0
            for gsz in GROUPS:
                nc.sync.dma_start(
                    x_sb[:, g0 : g0 + gsz, :], xt[:, g0 : g0 + gsz, :]
                )
                nc.sync.dma_start(
                    w_sb[:, g0 : g0 + gsz, :], wt[:, g0 : g0 + gsz, :]
                )
                g0 += gsz

            # s[b, (cl,o)] accumulated over all 256 k-chunks in one PSUM
            # tile; the route sum IS the PSUM accumulation.
            ps = psum_pool.tile([B, NCOL], f32)
            for kc in range(KC):
                nc.tensor.matmul(
                    ps,
                    x_sb[:, kc, :],
                    w_sb[:, kc, :],
                    start=(kc == 0),
                    stop=(kc == KC - 1),
                )

            # squash on [p=b, (cl, o)]: v = s * sqrt(sq) / (1 + sq).
            sv = qpool.tile([128, NCOL], f32, name="sv")
            nc.vector.tensor_scalar_mul(sv[:], ps[:], 1.0 / R)
            s2 = qpool.tile([128, CSH, O], f32, name="s2")
            nc.vector.tensor_mul(
                out=s2[:],
                in0=sv[:].rearrange("p (cl o) -> p cl o", o=O),
                in1=sv[:].rearrange("p (cl o) -> p cl o", o=O),
            )
            sq = qpool.tile([128, CSH], f32, name="sq")
            nc.vector.reduce_sum(sq[:], s2[:], axis=mybir.AxisListType.X)
            rt = qpool.tile([128, CSH], f32, name="rt")
            nc.scalar.sqrt(rt[:], sq[:])
            den = qpool.tile([128, CSH], f32, name="den")
            nc.vector.tensor_scalar_add(den[:], sq[:], 1.0)
            rec = qpool.tile([128, CSH], f32, name="rec")
            nc.vector.reciprocal(rec[:], den[:])
            fac = qpool.tile([128, CSH], f32, name="fac")
            nc.vector.tensor_mul(out=fac[:], in0=rt[:], in1=rec[:])
            v = qpool.tile([128, CSH, O], f32, name="v")
            nc.vector.tensor_tensor(
                v[:],
                sv[:].rearrange("p (cl o) -> p cl o", o=O),
                fac[:, :, None].to_broadcast((128, CSH, O)),
                mybir.AluOpType.mult,
            )
            nc.scalar.dma_start(out[:], v[:].rearrange("p cl o -> p (cl o)"))

    nc.compile()
    return nc


def _pack_inputs(x: np.ndarray, W: np.ndarray):
    """Per-core input layouts (host-side data movement + bf16 cast).

    Contraction chunk kc covers routes [8*kc, 8*kc+8); partition
    p = rp*16 + i with rp = r % 8 ... actually rp in [0,8), i in [0,16).
    """
    xb = x.astype(BF16)                               # [B, R, I]
    x_prep = np.ascontiguousarray(
        xb.reshape(B, KC, 8, I).transpose(2, 3, 1, 0)  # (rp, i, kc, b)
    ).reshape(128, KC, B)

    Wb = W[0].astype(BF16)                            # [R, C, O, I]
    in_maps = []
    for m in range(NCORES):
        Wm = Wb[:, m * CSH : (m + 1) * CSH]           # [R, 4, O, I]
        Wm = Wm.reshape(KC, 8, CSH, O, I)             # (kc, rp, cl, o, i)
        w_prep = np.ascontiguousarray(
            Wm.transpose(1, 4, 0, 2, 3)               # (rp, i, kc, cl, o)
        ).reshape(128, KC, NCOL)
        in_maps.append({"xt": x_prep, "wt": w_prep})
    return in_maps


_CACHED_NC = None


def _get_nc():
    global _CACHED_NC
    if _CACHED_NC is None:
        _CACHED_NC = _build_program()
    return _CACHED_NC


def kernel(x: np.ndarray, W: np.ndarray, _trace: bool = False):
    x = np.ascontiguousarray(np.asarray(x, dtype=np.float32))
    W = np.ascontiguousarray(np.asarray(W, dtype=np.float32))
    nc = _get_nc()
    in_maps = _pack_inputs(x, W)
    res = bass_utils.run_bass_kernel_spmd(
        nc, in_maps, core_ids=list(range(NCORES)), trace=_trace
    )
    out = np.concatenate(
        [res.results[m]["out"].reshape(B, CSH, O) for m in range(NCORES)],
        axis=1,
    ).reshape(B, C, O, 1)
    if _trace:
        return out, res
    return out


# revision 8
# speedup vs baseline: 2.0321x; 1.1156x over previous
"""Trainium2 Bass kernel for DigitCapsuleLayer (single routing iteration).

Math: with num_iterations == 1 the routing coefficients are uniform 1/R, so

    v[b,c,o] = squash( (1/R) * sum_{r,i} x[b,r,i] * W[0,r,c,o,i] )

one [B=128, K=32768] x [K=32768, N=1024] matmul + a tiny squash.  W is read
exactly once -> HBM-bound.

Sharding (8 cores): split the OUTPUT capsule dim C=32 so each core owns 4
capsules (128 columns) and computes them completely locally from its 8 MB
W slice + the full x (8 MB) -- both bf16 (tolerance 2e-2, bf16 lands at
~2e-3; the 1/R coefficient is folded into W host-side, an exact exponent
shift).  No collective: profiling the original K-sharded + AllToAll version
showed the ncfw machinery (entry-barrier rank skew + ~11 us setup + a 30 us
AllToAll for 256 KB) burning ~73 us after a 63 us near-roofline stream, and
any cross-core dependency also imports rank start-skew into the measured
span.  Trading 7 MB of replicated x for zero collectives wins by ~2.4x.

x and W are interleaved per 128-row contraction chunk in ONE packed DRAM
tensor [128 part, 256 kc, 128 x-cols | 128 w-cols] so the stream is a
single FIFO of large fully-contiguous DMAs (up to 20 KB/partition lines);
group sizes ramp up then shrink at the end because a group's matmuls can
only start once the whole group lands (a 52-chunk tail group cost 5.7 us of
post-stream drain in an earlier rev; the 4-chunk tail costs ~0.5 us).
Measured: the stream runs at the HBM-per-core limit (~358 GB/s fair-share,
up to ~400 GB/s when the stack-pair neighbor lags).

Raw bass (no TileContext), hand-scheduled with explicit semaphores: the
Tile framework's ~7 us preamble (entry barriers, ordering modes, pool
memsets) and ~3 us teardown drains shrink to ~1.5 us total -- the 10 stream
DMAs are the sync engine's first instructions.  Every RAW edge crosses or
follows an engine pipeline whose writes land asynchronously, so each gets
an explicit sem hop (what the Tile scheduler normally automates):

  sync:   dma g0..g9 (each then_inc sem_dma[g] 16) .. wait(sem_dve); out-DMA
  tensor: [wait sem_dma[g]>=16; matmuls of group g] x 10; last inc sem_pe
  scalar: warm sqrt (loads the ACT table off the critical path);
          wait(sem_pe); square(PSUM->s2); later sqrt(sq)->rt
  vector: reduce s2 -> sq; den=sq+1; rec=1/den; fac=rt*rec; v=ps*fac (bf16)
"""

import numpy as np
import ml_dtypes

import concourse.bacc as bacc
import concourse.bass as bass
import concourse.bass_utils as bass_utils
import concourse.mybir as mybir

B, R, C, I, O = 128, 2048, 32, 16, 32
NCORES = 8
CSH = C // NCORES
NCOL = CSH * O
KC = (R * I) // 128
FREE = B + NCOL
GROUPS = [16, 32, 40, 40, 40, 40, 32, 8, 4, 4]
assert sum(GROUPS) == KC

BF16 = ml_dtypes.bfloat16


def _build_program():
    nc = bacc.Bacc(
        "TRN2", target_bir_lowering=False, debug=False, num_devices=NCORES
    )
    f32 = mybir.dt.float32
    bf16 = mybir.dt.bfloat16

    xw = nc.dram_tensor("xw", [128, KC, FREE], bf16, kind="ExternalInput").ap()
    out = nc.dram_tensor("out", [B, NCOL], bf16, kind="ExternalOutput").ap()

    sem_dma = [nc.alloc_semaphore(f"sem_dma{i}") for i in range(len(GROUPS))]
    sem_pe = nc.alloc_semaphore("sem_pe")
    sem_act = nc.alloc_semaphore("sem_act")
    sem_dve = nc.alloc_semaphore("sem_dve")

    with (
        nc.sbuf_tensor("sb", [128, KC, FREE], bf16) as sb_h,
        nc.sbuf_tensor("s2", [128, NCOL], f32) as s2_h,
        nc.sbuf_tensor("sq", [128, CSH], f32) as sq_h,
        nc.sbuf_tensor("rt", [128, CSH], f32) as rt_h,
        nc.sbuf_tensor("den", [128, CSH], f32) as den_h,
        nc.sbuf_tensor("rec", [128, CSH], f32) as rec_h,
        nc.sbuf_tensor("fac", [128, CSH], f32) as fac_h,
        nc.sbuf_tensor("v", [128, NCOL], bf16) as v_h,
        nc.sbuf_tensor("warm", [1, 1], f32) as warm_h,
        nc.psum_tensor("ps", [B, NCOL], f32) as ps_h,
    ):
        sb, s2, sq = sb_h.ap(), s2_h.ap(), sq_h.ap()
        rt, den, rec, fac, v, warm = (
            rt_h.ap(), den_h.ap(), rec_h.ap(), fac_h.ap(), v_h.ap(), warm_h.ap()
        )
        ps = ps_h.ap()

        # Stream DMAs: very first instructions on the sync queue.
        g0 = 0
        for gi, gsz in enumerate(GROUPS):
            nc.sync.dma_start(
                sb[:, g0 : g0 + gsz, :], xw[:, g0 : g0 + gsz, :]
            ).then_inc(sem_dma[gi], 16)
            g0 += gsz

        # Matmul chain, paced by the DMA semaphore at group granularity.
        g0 = 0
        last_mm = None
        for gi, gsz in enumerate(GROUPS):
            nc.tensor.wait_ge(sem_dma[gi], 16)
            for kc in range(g0, g0 + gsz):
                last_mm = nc.tensor.matmul(
                    ps,
                    sb[:, kc, 0:B],
                    sb[:, kc, B:FREE],
                    start=(kc == 0),
                    stop=(kc == KC - 1),
                )
            g0 += gsz
        last_mm.then_inc(sem_pe, 1)

        # ACT: warm the Sqrt table way before it's needed, then the
        # square+accum (sum over o per capsule) and sqrt.
        sem_w = nc.alloc_semaphore("sem_w")
        nc.gpsimd.memset(warm, 0.0).then_inc(sem_w, 1)
        nc.scalar.wait_ge(sem_w, 1)
        nc.scalar.sqrt(warm, warm)
        # One plain square (beats accum_out: ACTIVATION_READ_ACCUMULATOR
        # costs ~290 ns per column group), then DVE reduces.
        sem_sq = nc.alloc_semaphore("sem_sq")
        sem_rq = nc.alloc_semaphore("sem_rq")
        nc.scalar.wait_ge(sem_pe, 1)
        nc.scalar.square(s2, ps).then_inc(sem_sq, 1)
        nc.vector.wait_ge(sem_sq, 1)
        nc.vector.reduce_sum(
            sq,
            s2.rearrange("p (cl o) -> p cl o", o=O),
            axis=mybir.AxisListType.X,
        ).then_inc(sem_rq, 1)
        nc.scalar.wait_ge(sem_rq, 1)
        nc.scalar.sqrt(rt, sq).then_inc(sem_act, 1)

        # DVE: den = sq+1; rec = 1/den; fac = rt*rec; v = s*fac.  The DVE
        # pipeline's writes land asynchronously, so every RAW edge gets a
        # sem hop (what the Tile scheduler normally automates).
        sem_v = nc.alloc_semaphore("sem_v")
        nc.vector.wait_ge(sem_act, 1)
        nc.vector.tensor_scalar_add(den, sq, 1.0).then_inc(sem_v, 1)
        nc.vector.wait_ge(sem_v, 1)
        nc.vector.reciprocal(rec, den).then_inc(sem_v, 1)
        nc.vector.wait_ge(sem_v, 2)
        nc.vector.tensor_mul(out=fac, in0=rt, in1=rec).then_inc(sem_v, 1)
        nc.vector.wait_ge(sem_v, 3)
        nc.vector.tensor_tensor(
            v.rearrange("p (cl o) -> p cl o", o=O),
            ps.rearrange("p (cl o) -> p cl o", o=O),
            fac[:, :, None].to_broadcast((128, CSH, O)),
            mybir.AluOpType.mult,
        ).then_inc(sem_dve, 1)

        # Output DMA rides the (now idle) sync ring.  Every raw dma_start
        # needs a completion semaphore (HWDGE codegen asserts otherwise).
        sem_out = nc.alloc_semaphore("sem_out")
        nc.sync.wait_ge(sem_dve, 1)
        nc.sync.dma_start(out, v).then_inc(sem_out, 16)
        nc.sync.wait_ge(sem_out, 16)

    nc.compile()
    return nc


# Host-side packing identical to kernel.py v3.
def _pack_inputs(x: np.ndarray, W: np.ndarray):
    xb = x.astype(BF16)
    x_prep = np.ascontiguousarray(
        xb.reshape(B, KC, 8, I).transpose(2, 3, 1, 0)
    ).reshape(128, KC, B)
    Wb = (W[0] * (1.0 / R)).astype(BF16)
    in_maps = []
    for m in range(NCORES):
        Wm = Wb[:, m * CSH : (m + 1) * CSH]
        Wm = Wm.reshape(KC, 8, CSH, O, I)
        w_prep = np.ascontiguousarray(
            Wm.transpose(1, 4, 0, 2, 3)
        ).reshape(128, KC, NCOL)
        in_maps.append({"xw": np.concatenate([x_prep, w_prep], axis=2)})
    return in_maps


_CACHED_NC = None


def _get_nc():
    global _CACHED_NC
    if _CACHED_NC is None:
        _CACHED_NC = _build_program()
    return _CACHED_NC


def kernel(x: np.ndarray, W: np.ndarray, _trace: bool = False):
    x = np.ascontiguousarray(np.asarray(x, dtype=np.float32))
    W = np.ascontiguousarray(np.asarray(W, dtype=np.float32))
    nc = _get_nc()
    in_maps = _pack_inputs(x, W)
    res = bass_utils.run_bass_kernel_spmd(
        nc, in_maps, core_ids=list(range(NCORES)), trace=_trace
    )
    out = np.concatenate(
        [np.asarray(res.results[m]["out"], dtype=np.float32).reshape(B, CSH, O)
         for m in range(NCORES)],
        axis=1,
    ).reshape(B, C, O, 1)
    if _trace:
        return out, res
    return out


# revision 10
# speedup vs baseline: 2.0673x; 1.0173x over previous
"""Trainium2 Bass kernel for DigitCapsuleLayer (single routing iteration).

Math: with num_iterations == 1 the routing coefficients are uniform 1/R, so

    v[b,c,o] = squash( (1/R) * sum_{r,i} x[b,r,i] * W[0,r,c,o,i] )

one [B=128, K=32768] x [K=32768, N=1024] matmul + a tiny squash.  W is read
exactly once -> HBM-bound.

Sharding (8 cores): split the OUTPUT capsule dim C=32 so each core owns 4
capsules (128 columns) and computes them completely locally from its 8 MB
W slice + the full x (8 MB) -- both bf16 (tolerance 2e-2, bf16 lands at
~2e-3; the 1/R coefficient is folded into W host-side, an exact exponent
shift).  No collective: profiling the original K-sharded + AllToAll version
showed the ncfw machinery (entry-barrier rank skew + ~11 us setup + a 30 us
AllToAll for 256 KB) burning ~73 us after a 63 us near-roofline stream, and
any cross-core dependency also imports rank start-skew into the measured
span.  Trading 7 MB of replicated x for zero collectives wins by ~2.4x.

x and W are interleaved per 128-row contraction chunk in ONE packed DRAM
tensor [128 part, 256 kc, 128 x-cols | 128 w-cols] so the stream is a
single FIFO of large fully-contiguous DMAs (up to 20 KB/partition lines);
group sizes ramp up then shrink at the end because a group's matmuls can
only start once the whole group lands (a 52-chunk tail group cost 5.7 us of
post-stream drain in an earlier rev; the 4-chunk tail costs ~0.5 us).
Measured: the stream runs at the HBM-per-core limit (~358 GB/s fair-share,
up to ~400 GB/s when the stack-pair neighbor lags).

Raw bass (no TileContext), hand-scheduled with explicit semaphores: the
Tile framework's ~7 us preamble (entry barriers, ordering modes, pool
memsets) and ~3 us teardown drains shrink to ~1.5 us total -- the 10 stream
DMAs are the sync engine's first instructions.  Every RAW edge crosses or
follows an engine pipeline whose writes land asynchronously, so each gets
an explicit sem hop (what the Tile scheduler normally automates):

  sync:   dma g0..g9 (each then_inc sem_dma[g] 16) .. wait(sem_dve); out-DMA
  tensor: [wait sem_dma[g]>=16; matmuls of group g] x 10; last inc sem_pe
  scalar: warm sqrt (loads the ACT table off the critical path);
          wait(sem_pe); square(PSUM->s2); later sqrt(sq)->rt
  vector: reduce s2 -> sq; den=sq+1; rec=1/den; fac=rt*rec; v=ps*fac (bf16)
"""

import numpy as np
import ml_dtypes

import concourse.bacc as bacc
import concourse.bass as bass
import concourse.bass_utils as bass_utils
import concourse.mybir as mybir

B, R, C, I, O = 128, 2048, 32, 16, 32
NCORES = 8
CSH = C // NCORES
NCOL = CSH * O
KC = (R * I) // 128
FREE = B + NCOL
GROUPS = [16, 32, 40, 40, 40, 40, 32, 8, 4, 4]
assert sum(GROUPS) == KC

BF16 = ml_dtypes.bfloat16


def _build_program():
    nc = bacc.Bacc(
        "TRN2", target_bir_lowering=False, debug=False, num_devices=NCORES
    )
    f32 = mybir.dt.float32
    bf16 = mybir.dt.bfloat16

    xw = nc.dram_tensor("xw", [128, KC, FREE], bf16, kind="ExternalInput").ap()
    out = nc.dram_tensor("out", [B, NCOL], bf16, kind="ExternalOutput").ap()

    sem_dma = [nc.alloc_semaphore(f"sem_dma{i}") for i in range(len(GROUPS))]
    sem_pe = nc.alloc_semaphore("sem_pe")
    sem_act = nc.alloc_semaphore("sem_act")
    sem_dve = nc.alloc_semaphore("sem_dve")

    with (
        nc.sbuf_tensor("sb", [128, KC, FREE], bf16) as sb_h,
        nc.sbuf_tensor("s2", [128, NCOL], f32) as s2_h,
        nc.sbuf_tensor("sq", [128, CSH], f32) as sq_h,
        nc.sbuf_tensor("rt", [128, CSH], f32) as rt_h,
        nc.sbuf_tensor("den", [128, CSH], f32) as den_h,
        nc.sbuf_tensor("rec", [128, CSH], f32) as rec_h,
        nc.sbuf_tensor("fac", [128, CSH], f32) as fac_h,
        nc.sbuf_tensor("v", [128, NCOL], bf16) as v_h,
        nc.sbuf_tensor("warm", [1, 1], f32) as warm_h,
        nc.psum_tensor("ps", [B, NCOL], f32) as ps_h,
    ):
        sb, s2, sq = sb_h.ap(), s2_h.ap(), sq_h.ap()
        rt, den, rec, fac, v, warm = (
            rt_h.ap(), den_h.ap(), rec_h.ap(), fac_h.ap(), v_h.ap(), warm_h.ap()
        )
        ps = ps_h.ap()

        # Stream DMAs: very first instructions on the sync queue.
        g0 = 0
        for gi, gsz in enumerate(GROUPS):
            nc.sync.dma_start(
                sb[:, g0 : g0 + gsz, :], xw[:, g0 : g0 + gsz, :]
            ).then_inc(sem_dma[gi], 16)
            g0 += gsz

        # Matmul chain, paced by the DMA semaphore at group granularity.
        g0 = 0
        last_mm = None
        for gi, gsz in enumerate(GROUPS):
            nc.tensor.wait_ge(sem_dma[gi], 16)
            for kc in range(g0, g0 + gsz):
                last_mm = nc.tensor.matmul(
                    ps,
                    sb[:, kc, 0:B],
                    sb[:, kc, B:FREE],
                    start=(kc == 0),
                    stop=(kc == KC - 1),
                )
            g0 += gsz
        last_mm.then_inc(sem_pe, 1)

        # ACT: warm the Sqrt table way before it's needed, then the
        # square+accum (sum over o per capsule) and sqrt.
        sem_w = nc.alloc_semaphore("sem_w")
        nc.gpsimd.memset(warm, 0.0).then_inc(sem_w, 1)
        nc.scalar.wait_ge(sem_w, 1)
        nc.scalar.sqrt(warm, warm)
        # Squash pipelined in two capsule-halves (columns 0:64 and 64:128)
        # so the first half's output DMA fires while the second half is
        # still on the vector engine: plain square (beats accum_out --
        # ACTIVATION_READ_ACCUMULATOR costs ~290 ns per group), DVE
        # reduce, ACT sqrt, DVE finish.  Half-chains are interleaved on
        # DVE so each RAW sem hop hides under the other half's op.
        HC = CSH // 2           # 2 capsules per half
        HN = NCOL // 2          # 64 columns per half
        sem_sq = nc.alloc_semaphore("sem_sq")
        sem_rq = nc.alloc_semaphore("sem_rq")
        sem_v = nc.alloc_semaphore("sem_v")
        sem_out = nc.alloc_semaphore("sem_out")

        def half(t, h):
            return t[:, h * HN : (h + 1) * HN]

        def halfc(t, h):
            return t[:, h * HC : (h + 1) * HC]

        nc.scalar.wait_ge(sem_pe, 1)
        for h in range(2):
            nc.scalar.square(half(s2, h), half(ps, h)).then_inc(sem_sq, 1)
        for h in range(2):
            nc.vector.wait_ge(sem_sq, h + 1)
            nc.vector.reduce_sum(
                halfc(sq, h),
                half(s2, h).rearrange("p (cl o) -> p cl o", o=O),
                axis=mybir.AxisListType.X,
            ).then_inc(sem_rq, 1)
        for h in range(2):
            nc.scalar.wait_ge(sem_rq, h + 1)
            nc.scalar.sqrt(halfc(rt, h), halfc(sq, h)).then_inc(sem_act, 1)
        # DVE: den = sq+1; rec = 1/den; fac = rt*rec; v = s*fac, halves
        # interleaved (sem_v counts: den0,den1,rec0,rec1,fac0,fac1).
        for h in range(2):
            nc.vector.wait_ge(sem_rq, h + 1)
            nc.vector.tensor_scalar_add(
                halfc(den, h), halfc(sq, h), 1.0
            ).then_inc(sem_v, 1)
        for h in range(2):
            nc.vector.wait_ge(sem_v, h + 1)
            nc.vector.reciprocal(halfc(rec, h), halfc(den, h)).then_inc(sem_v, 1)
        for h in range(2):
            nc.vector.wait_ge(sem_act, h + 1)
            nc.vector.wait_ge(sem_v, h + 3)
            nc.vector.tensor_mul(
                out=halfc(fac, h), in0=halfc(rt, h), in1=halfc(rec, h)
            ).then_inc(sem_v, 1)
        for h in range(2):
            nc.vector.wait_ge(sem_v, h + 5)
            nc.vector.tensor_tensor(
                half(v, h).rearrange("p (cl o) -> p cl o", o=O),
                half(ps, h).rearrange("p (cl o) -> p cl o", o=O),
                halfc(fac, h)[:, :, None].to_broadcast((128, HC, O)),
                mybir.AluOpType.mult,
            ).then_inc(sem_dve, 1)
        # Two output DMAs on the (now idle) sync ring; the first hides
        # under the second half's DVE work.  Every raw dma_start needs a
        # completion semaphore (HWDGE codegen asserts otherwise).
        for h in range(2):
            nc.sync.wait_ge(sem_dve, h + 1)
            nc.sync.dma_start(half(out, h), half(v, h)).then_inc(sem_out, 16)
        nc.sync.wait_ge(sem_out, 32)

    nc.compile()
    return nc


# Host-side packing identical to kernel.py v3.
def _pack_inputs(x: np.ndarray, W: np.ndarray):
    xb = x.astype(BF16)
    x_prep = np.ascontiguousarray(
        xb.reshape(B, KC, 8, I).transpose(2, 3, 1, 0)
    ).reshape(128, KC, B)
    Wb = (W[0] * (1.0 / R)).astype(BF16)
    in_maps = []
    for m in range(NCORES):
        Wm = Wb[:, m * CSH : (m + 1) * CSH]
        Wm = Wm.reshape(KC, 8, CSH, O, I)
        w_prep = np.ascontiguousarray(
            Wm.transpose(1, 4, 0, 2, 3)
        ).reshape(128, KC, NCOL)
        in_maps.append({"xw": np.concatenate([x_prep, w_prep], axis=2)})
    return in_maps


_CACHED_NC = None


def _get_nc():
    global _CACHED_NC
    if _CACHED_NC is None:
        _CACHED_NC = _build_program()
    return _CACHED_NC


def kernel(x: np.ndarray, W: np.ndarray, _trace: bool = False):
    x = np.ascontiguousarray(np.asarray(x, dtype=np.float32))
    W = np.ascontiguousarray(np.asarray(W, dtype=np.float32))
    nc = _get_nc()
    in_maps = _pack_inputs(x, W)
    res = bass_utils.run_bass_kernel_spmd(
        nc, in_maps, core_ids=list(range(NCORES)), trace=_trace
    )
    out = np.concatenate(
        [np.asarray(res.results[m]["out"], dtype=np.float32).reshape(B, CSH, O)
         for m in range(NCORES)],
        axis=1,
    ).reshape(B, C, O, 1)
    if _trace:
        return out, res
    return out


# revision 11
# speedup vs baseline: 2.4165x; 1.1689x over previous
"""Trainium2 Bass kernel for DigitCapsuleLayer (single routing iteration).

Math: with num_iterations == 1 the routing coefficients are uniform 1/R, so

    v[b,c,o] = squash( (1/R) * sum_{r,i} x[b,r,i] * W[0,r,c,o,i] )

one [B=128, K=32768] x [K=32768, N=1024] matmul + a tiny squash.  W is read
exactly once -> HBM-bound.

Sharding (8 cores): split the OUTPUT capsule dim C=32 so each core owns 4
capsules (128 columns) and computes them completely locally from its 8 MB
W slice + the full x (8 MB) -- both bf16 (tolerance 2e-2, bf16 lands at
~2e-3; the 1/R coefficient is folded into W host-side, an exact exponent
shift).  No collective: profiling the original K-sharded + AllToAll version
showed the ncfw machinery (entry-barrier rank skew + ~11 us setup + a 30 us
AllToAll for 256 KB) burning ~73 us after a 63 us near-roofline stream, and
any cross-core dependency also imports rank start-skew into the measured
span.  Trading 7 MB of replicated x for zero collectives wins by ~2.4x.

x and W are interleaved per 128-row contraction chunk in ONE packed DRAM
tensor [128 part, 256 kc, 128 x-cols | 128 w-cols] so the stream is a
single FIFO of large fully-contiguous DMAs (up to 20 KB/partition lines);
group sizes ramp up then shrink at the end because a group's matmuls can
only start once the whole group lands (a 52-chunk tail group cost 5.7 us of
post-stream drain in an earlier rev; the 4-chunk tail costs ~0.5 us).
Measured: the stream runs at the HBM-per-core limit (~358 GB/s fair-share,
up to ~400 GB/s when the stack-pair neighbor lags).

Raw bass (no TileContext), hand-scheduled with explicit semaphores: the
Tile framework's ~7 us preamble (entry barriers, ordering modes, pool
memsets) and ~3 us teardown drains shrink to ~1.5 us total -- the 10 stream
DMAs are the sync engine's first instructions.  Every RAW edge crosses or
follows an engine pipeline whose writes land asynchronously, so each gets
an explicit sem hop (what the Tile scheduler normally automates):

  sync:   dma g0..g9 (each then_inc sem_dma[g] 16) .. wait(sem_dve); out-DMA
  tensor: [wait sem_dma[g]>=16; matmuls of group g] x 10; last inc sem_pe
  scalar: warm sqrt (loads the ACT table off the critical path);
          wait(sem_pe); square(PSUM->s2); later sqrt(sq)->rt
  vector: reduce s2 -> sq; den=sq+1; rec=1/den; fac=rt*rec; v=ps*fac (bf16)
"""

import numpy as np
import ml_dtypes

import concourse.bacc as bacc
import concourse.bass as bass
import concourse.bass_utils as bass_utils
import concourse.mybir as mybir

B, R, C, I, O = 128, 2048, 32, 16, 32
NCORES = 8
CSH = C // NCORES
NCOL = CSH * O
KC = (R * I) // 128
FREE = B + NCOL
GROUPS = [16, 32, 40, 40, 40, 40, 32, 8, 4, 4]
assert sum(GROUPS) == KC

BF16 = ml_dtypes.bfloat16


def _build_program():
    nc = bacc.Bacc(
        "TRN2", target_bir_lowering=False, debug=False, num_devices=NCORES
    )
    f32 = mybir.dt.float32
    bf16 = mybir.dt.bfloat16

    xw = nc.dram_tensor("xw", [128, KC, FREE], bf16, kind="ExternalInput").ap()
    out = nc.dram_tensor("out", [B, NCOL], bf16, kind="ExternalOutput").ap()

    sem_dma = [nc.alloc_semaphore(f"sem_dma{i}") for i in range(len(GROUPS))]
    sem_pe = nc.alloc_semaphore("sem_pe")
    sem_act = nc.alloc_semaphore("sem_act")
    sem_dve = nc.alloc_semaphore("sem_dve")

    with (
        nc.sbuf_tensor("sb", [128, KC, FREE], bf16) as sb_h,
        nc.sbuf_tensor("s2", [128, NCOL], f32) as s2_h,
        nc.sbuf_tensor("sq", [128, CSH], f32) as sq_h,
        nc.sbuf_tensor("rt", [128, CSH], f32) as rt_h,
        nc.sbuf_tensor("den", [128, CSH], f32) as den_h,
        nc.sbuf_tensor("rec", [128, CSH], f32) as rec_h,
        nc.sbuf_tensor("fac", [128, CSH], f32) as fac_h,
        nc.sbuf_tensor("v", [128, NCOL], bf16) as v_h,
        nc.sbuf_tensor("warm", [1, 1], f32) as warm_h,
        nc.psum_tensor("ps", [B, NCOL], f32) as ps_h,
    ):
        sb, s2, sq = sb_h.ap(), s2_h.ap(), sq_h.ap()
        rt, den, rec, fac, v, warm = (
            rt_h.ap(), den_h.ap(), rec_h.ap(), fac_h.ap(), v_h.ap(), warm_h.ap()
        )
        ps = ps_h.ap()

        # Stream DMAs: very first instructions on the sync queue.
        g0 = 0
        for gi, gsz in enumerate(GROUPS):
            nc.sync.dma_start(
                sb[:, g0 : g0 + gsz, :], xw[:, g0 : g0 + gsz, :]
            ).then_inc(sem_dma[gi], 16)
            g0 += gsz

        # Matmul chain, paced by the DMA semaphore at group granularity.
        g0 = 0
        last_mm = None
        for gi, gsz in enumerate(GROUPS):
            nc.tensor.wait_ge(sem_dma[gi], 16)
            for kc in range(g0, g0 + gsz):
                last_mm = nc.tensor.matmul(
                    ps,
                    sb[:, kc, 0:B],
                    sb[:, kc, B:FREE],
                    start=(kc == 0),
                    stop=(kc == KC - 1),
                )
            g0 += gsz
        last_mm.then_inc(sem_pe, 1)

        # ACT: warm the Sqrt table way before it's needed, then the
        # square+accum (sum over o per capsule) and sqrt.
        sem_w = nc.alloc_semaphore("sem_w")
        nc.gpsimd.memset(warm, 0.0).then_inc(sem_w, 1)
        nc.scalar.wait_ge(sem_w, 1)
        nc.scalar.sqrt(warm, warm)
        # Squash pipelined in two capsule-halves (columns 0:64 and 64:128)
        # so the first half's output DMA fires while the second half is
        # still on the vector engine: plain square (beats accum_out --
        # ACTIVATION_READ_ACCUMULATOR costs ~290 ns per group), DVE
        # reduce, ACT sqrt, DVE finish.  Half-chains are interleaved on
        # DVE so each RAW sem hop hides under the other half's op.
        HC = CSH // 2           # 2 capsules per half
        HN = NCOL // 2          # 64 columns per half
        sem_sq = nc.alloc_semaphore("sem_sq")
        sem_rq = nc.alloc_semaphore("sem_rq")
        sem_v = nc.alloc_semaphore("sem_v")
        sem_out = nc.alloc_semaphore("sem_out")

        def half(t, h):
            return t[:, h * HN : (h + 1) * HN]

        def halfc(t, h):
            return t[:, h * HC : (h + 1) * HC]

        nc.scalar.wait_ge(sem_pe, 1)
        for h in range(2):
            nc.scalar.square(half(s2, h), half(ps, h)).then_inc(sem_sq, 1)
        for h in range(2):
            nc.vector.wait_ge(sem_sq, h + 1)
            nc.vector.reduce_sum(
                halfc(sq, h),
                half(s2, h).rearrange("p (cl o) -> p cl o", o=O),
                axis=mybir.AxisListType.X,
            ).then_inc(sem_rq, 1)
        for h in range(2):
            nc.scalar.wait_ge(sem_rq, h + 1)
            nc.scalar.sqrt(halfc(rt, h), halfc(sq, h)).then_inc(sem_act, 1)
        # DVE: den = sq+1; rec = 1/den; fac = rt*rec; v = s*fac, halves
        # interleaved (sem_v counts: den0,den1,rec0,rec1,fac0,fac1).
        for h in range(2):
            nc.vector.wait_ge(sem_rq, h + 1)
            nc.vector.tensor_scalar_add(
                halfc(den, h), halfc(sq, h), 1.0
            ).then_inc(sem_v, 1)
        for h in range(2):
            nc.vector.wait_ge(sem_v, h + 1)
            nc.vector.reciprocal(halfc(rec, h), halfc(den, h)).then_inc(sem_v, 1)
        for h in range(2):
            nc.vector.wait_ge(sem_act, h + 1)
            nc.vector.wait_ge(sem_v, h + 3)
            nc.vector.tensor_mul(
                out=halfc(fac, h), in0=halfc(rt, h), in1=halfc(rec, h)
            ).then_inc(sem_v, 1)
        for h in range(2):
            nc.vector.wait_ge(sem_v, h + 5)
            nc.vector.tensor_tensor(
                half(v, h).rearrange("p (cl o) -> p cl o", o=O),
                half(ps, h).rearrange("p (cl o) -> p cl o", o=O),
                halfc(fac, h)[:, :, None].to_broadcast((128, HC, O)),
                mybir.AluOpType.mult,
            ).then_inc(sem_dve, 1)
        # Two output DMAs on separate HWDGE rings (sync + the now-idle
        # scalar ring) so their completion receipts overlap; the first
        # also hides under the second half's DVE work.  Every raw
        # dma_start needs a completion semaphore (HWDGE codegen asserts
        # otherwise).
        nc.sync.wait_ge(sem_dve, 1)
        nc.sync.dma_start(half(out, 0), half(v, 0)).then_inc(sem_out, 16)
        nc.scalar.wait_ge(sem_dve, 2)
        nc.scalar.dma_start(half(out, 1), half(v, 1)).then_inc(sem_out, 16)
        nc.sync.wait_ge(sem_out, 32)

    nc.compile()
    return nc


# Host-side packing identical to kernel.py v3.
def _pack_inputs(x: np.ndarray, W: np.ndarray):
    xb = x.astype(BF16)
    x_prep = np.ascontiguousarray(
        xb.reshape(B, KC, 8, I).transpose(2, 3, 1, 0)
    ).reshape(128, KC, B)
    Wb = (W[0] * (1.0 / R)).astype(BF16)
    in_maps = []
    for m in range(NCORES):
        Wm = Wb[:, m * CSH : (m + 1) * CSH]
        Wm = Wm.reshape(KC, 8, CSH, O, I)
        w_prep = np.ascontiguousarray(
            Wm.transpose(1, 4, 0, 2, 3)
        ).reshape(128, KC, NCOL)
        in_maps.append({"xw": np.concatenate([x_prep, w_prep], axis=2)})
    return in_maps


_CACHED_NC = None


def _get_nc():
    global _CACHED_NC
    if _CACHED_NC is None:
        _CACHED_NC = _build_program()
    return _CACHED_NC


def kernel(x: np.ndarray, W: np.ndarray, _trace: bool = False):
    x = np.ascontiguousarray(np.asarray(x, dtype=np.float32))
    W = np.ascontiguousarray(np.asarray(W, dtype=np.float32))
    nc = _get_nc()
    in_maps = _pack_inputs(x, W)
    res = bass_utils.run_bass_kernel_spmd(
        nc, in_maps, core_ids=list(range(NCORES)), trace=_trace
    )
    out = np.concatenate(
        [np.asarray(res.results[m]["out"], dtype=np.float32).reshape(B, CSH, O)
         for m in range(NCORES)],
        axis=1,
    ).reshape(B, C, O, 1)
    if _trace:
        return out, res
    return out
